# revision 8
# baseline (speedup 1.0000x reference)
"""Compact-prefix attention (nn_Attention_16234976379516) on 8 TRN2 NeuronCores.

Math per (b, h) pair:
    S = (Q @ K^T) * scale          [T, L]
    S[:, :Lc] += beta              (bias on compacted prefix)
    S = where(mask, S, -inf)       (mask folded into bias host-side)
    O = softmax(S, -1) @ V         [T, D]

Device formulation (transposed scores, no on-chip transposes):
    E^T[l, t] = exp(scale * (K Q^T)[l, t] + bias[l])   # PE matmul + ScalarE Exp
    [O*denom | denom] = sum_lc E^T_lc.T @ [V_lc | 1]   # PE accumulation, ones
                                                       # column yields softmax
                                                       # denominator for free
    O = (O*denom) * (1/denom)                          # DVE epilogue

Sharding: (B,H) flattened to 128 pairs, 16 per core (pure H/batch split, no
cross-device communication). Host pre-transposes Q/K per head, pre-tiles V
with an appended ones column, folds beta+mask into a [128, n_lc] bias tile,
and casts operands to bf16 (fp32 PSUM accumulation throughout).
"""

import numpy as np

B, H, T, L, LC, D = 4, 32, 256, 4096, 2048, 128
NCORES = 8
G = B * H                  # 128 (b,h) pairs
GPC = G // NCORES          # 16 pairs per core
N_LC = L // 128            # 32 l-chunks of 128
N_TC = T // 128            # 2 t-chunks of 128
SCALE = 1.0 / float(np.sqrt(D))

# DMA split factors (more concurrent DMA queues in flight)
KT_SPLIT = 8
V1_SPLIT = 4

_NC_CACHE = {}


def build_nc(n_heads=GPC, n_lc=N_LC, n_tc=N_TC):
    """Build the single-core Bass program (run SPMD on all 8 cores)."""
    from contextlib import ExitStack

    import concourse.bacc as bacc
    import concourse.mybir as mybir
    import concourse.tile as tile
    from concourse.bass import ts

    bf16 = mybir.dt.bfloat16
    f32 = mybir.dt.float32
    L_ = n_lc * 128
    T_ = n_tc * 128
    # l-chunks of scores per wide ACT instruction (one exp over [128, ACT_W*T_])
    ACT_W = 4
    assert n_lc % ACT_W == 0

    nc = bacc.Bacc("TRN2", target_bir_lowering=False, debug=False)
    qT_d = nc.dram_tensor("qT", [n_heads, 128, T_], bf16, kind="ExternalInput").ap()
    kT_d = nc.dram_tensor("kT", [n_heads, 128, L_], bf16, kind="ExternalInput").ap()
    # v1[g, p, lc, d]: e^bias[l] * (V row l | 1), l = lc*128 + p
    v1_d = nc.dram_tensor(
        "v1", [n_heads, 128, n_lc, 129], bf16, kind="ExternalInput"
    ).ap()
    out_d = nc.dram_tensor("out", [n_heads, n_tc, 128, 128], f32, kind="ExternalOutput").ap()

    with tile.TileContext(nc) as tc:
        with ExitStack() as ctx:
            in_pool = ctx.enter_context(tc.tile_pool(name="in_pool", bufs=4))
            e_pool = ctx.enter_context(tc.tile_pool(name="e_pool", bufs=3))
            ep_pool = ctx.enter_context(tc.tile_pool(name="ep_pool", bufs=4))
            s_pool = ctx.enter_context(tc.tile_pool(name="s_pool", bufs=3, space="PSUM"))
            o_pool = ctx.enter_context(tc.tile_pool(name="o_pool", bufs=2, space="PSUM"))

            for g in range(n_heads):
                qT = in_pool.tile([128, T_], bf16, tag="qT", name="qT_sb")
                nc.sync.dma_start(out=qT, in_=qT_d[g])
                kT = in_pool.tile([128, L_], bf16, tag="kT", name="kT_sb")
                for c in range(KT_SPLIT):
                    w = L_ // KT_SPLIT
                    nc.sync.dma_start(
                        out=kT[:, c * w : (c + 1) * w],
                        in_=kT_d[g, :, c * w : (c + 1) * w],
                    )
                v1 = in_pool.tile([128, n_lc, 129], bf16, tag="v1", name="v1_sb")
                for c in range(V1_SPLIT):
                    w = n_lc // V1_SPLIT
                    nc.gpsimd.dma_start(
                        out=v1[:, c * w : (c + 1) * w, :],
                        in_=v1_d[g, :, c * w : (c + 1) * w, :],
                    )

                # Stage 1: E^T[l, t] = exp(scale * (K Q^T)[l, t]), bf16.
                # ACT_W score matmuls land in one 2-bank PSUM tile; a single
                # wide Exp covers them all (amortizes ACT fixed overhead).
                e = e_pool.tile([128, n_lc, T_], bf16, tag="e", name="e_sb")
                for a in range(n_lc // ACT_W):
                    s = s_pool.tile([128, ACT_W, T_], f32, tag="s", name="s_ps")
                    for j in range(ACT_W):
                        lc = a * ACT_W + j
                        nc.tensor.matmul(
                            s[:, j, :],
                            lhsT=kT[:, ts(lc, 128)],
                            rhs=qT,
                            start=True,
                            stop=True,
                        )
                    nc.scalar.activation(
                        out=e[:, a * ACT_W : (a + 1) * ACT_W, :],
                        in_=s,
                        func=mybir.ActivationFunctionType.Exp,
                        scale=SCALE,
                    )

                # Stage 2: accumulate [O*denom | denom] over l-chunks
                os_ = []
                for tci in range(n_tc):
                    o = o_pool.tile([128, 129], f32, tag="o", name="o_ps")
                    os_.append(o)
                for lc in range(n_lc):
                    for tci in range(n_tc):
                        nc.tensor.matmul(
                            os_[tci],
                            lhsT=e[:, lc, ts(tci, 128)],
                            rhs=v1[:, lc, :],
                            start=(lc == 0),
                            stop=(lc == n_lc - 1),
                        )

                # Epilogue: O = (O*denom) / denom
                for tci in range(n_tc):
                    recip = ep_pool.tile([128, 1], f32, tag="recip", name="recip_sb")
                    nc.vector.reciprocal(recip, os_[tci][:, 128:129])
                    ob = ep_pool.tile([128, 128], f32, tag="ob", name="ob_sb")
                    nc.vector.tensor_scalar_mul(ob, os_[tci][:, 0:128], recip)
                    nc.sync.dma_start(out=out_d[g, tci], in_=ob)

    nc.compile()
    return nc


def make_core_inputs(q, k, v, beta, attn_mask):
    """Host prep: fold mask+beta into bias, transpose/tile/cast, shard 8 ways.

    Returns list of 8 in_maps (one per core)."""
    import ml_dtypes

    bf16 = ml_dtypes.bfloat16

    qf = np.ascontiguousarray(q, np.float32).reshape(G, T, D)
    kf = np.ascontiguousarray(k, np.float32).reshape(G, L, D)
    vf = np.ascontiguousarray(v, np.float32).reshape(G, L, D)

    bias = np.zeros((G, L), np.float32)
    bias[:, :LC] = np.asarray(beta, np.float32).reshape(G, LC)
    mask = np.asarray(attn_mask).reshape(G, L)
    # exp(s + b) = exp(s) * e^b: fold e^bias into the [V | 1] operand so the
    # device exp needs no per-partition bias (enables wide ACT tiles). A
    # masked-out l gets e^-inf = 0, zeroing its numerator+denominator terms.
    ebias = np.where(mask, np.exp(bias), np.float32(0.0))

    in_maps = []
    for i in range(NCORES):
        sl = slice(i * GPC, (i + 1) * GPC)
        qT = np.ascontiguousarray(qf[sl].transpose(0, 2, 1)).astype(bf16)
        kT = np.ascontiguousarray(kf[sl].transpose(0, 2, 1)).astype(bf16)
        v1 = np.empty((GPC, L, D + 1), np.float32)
        v1[..., :D] = vf[sl]
        v1[..., D] = 1.0
        v1 *= ebias[sl, :, None]
        v1 = v1.reshape(GPC, N_LC, 128, D + 1).transpose(0, 2, 1, 3)
        in_maps.append(
            {"qT": qT, "kT": kT, "v1": np.ascontiguousarray(v1.astype(bf16))}
        )
    return in_maps


def run_spmd(in_maps, trace=False):
    from concourse import bass_utils

    if "nc" not in _NC_CACHE:
        _NC_CACHE["nc"] = build_nc()
    nc = _NC_CACHE["nc"]
    return bass_utils.run_bass_kernel_spmd(
        nc, in_maps, core_ids=list(range(NCORES)), trace=trace
    )


def kernel(q, k, v, beta, attn_mask):
    res = run_spmd(make_core_inputs(q, k, v, beta, attn_mask))
    out = np.empty((G, T, D), np.float32)
    for i in range(NCORES):
        out[i * GPC : (i + 1) * GPC] = res.results[i]["out"].reshape(GPC, T, D)
    return out.reshape(B, H, T, D)


# revision 10
# speedup vs baseline: 1.1216x; 1.1216x over previous
"""Compact-prefix attention (nn_Attention_16234976379516) on 8 TRN2 NeuronCores.

Math per (b, h) pair:
    S = (Q @ K^T) * scale          [T, L]
    S[:, :Lc] += beta              (bias on compacted prefix)
    S = where(mask, S, -inf)       (mask folded into bias host-side)
    O = softmax(S, -1) @ V         [T, D]

Device formulation (transposed scores, no on-chip transposes):
    E^T[l, t] = exp(scale * (K Q^T)[l, t] + bias[l])   # PE matmul + ScalarE Exp
    [O*denom | denom] = sum_lc E^T_lc.T @ [V_lc | 1]   # PE accumulation, ones
                                                       # column yields softmax
                                                       # denominator for free
    O = (O*denom) * (1/denom)                          # DVE epilogue

Sharding: (B,H) flattened to 128 pairs, 16 per core (pure H/batch split, no
cross-device communication). Host pre-transposes Q/K per head, pre-tiles V
with an appended ones column, folds beta+mask into a [128, n_lc] bias tile,
and casts operands to bf16 (fp32 PSUM accumulation throughout).
"""

import numpy as np

B, H, T, L, LC, D = 4, 32, 256, 4096, 2048, 128
NCORES = 8
G = B * H                  # 128 (b,h) pairs
GPC = G // NCORES          # 16 pairs per core
N_LC = L // 128            # 32 l-chunks of 128
N_TC = T // 128            # 2 t-chunks of 128
SCALE = 1.0 / float(np.sqrt(D))

# DMA split factors (more concurrent DMA queues in flight)
KT_SPLIT = 4
V1_SPLIT = 4

_NC_CACHE = {}


def build_nc(n_heads=GPC, n_lc=N_LC, n_tc=N_TC):
    """Build the single-core Bass program (run SPMD on all 8 cores)."""
    from contextlib import ExitStack

    import concourse.bacc as bacc
    import concourse.mybir as mybir
    import concourse.tile as tile
    from concourse.bass import ts

    bf16 = mybir.dt.bfloat16
    f32 = mybir.dt.float32
    L_ = n_lc * 128
    T_ = n_tc * 128
    # l-chunks of scores per wide ACT instruction (one exp over [128, ACT_W*T_])
    ACT_W = 4
    assert n_lc % ACT_W == 0

    nc = bacc.Bacc("TRN2", target_bir_lowering=False, debug=False)
    qT_d = nc.dram_tensor("qT", [n_heads, 128, T_], bf16, kind="ExternalInput").ap()
    kT_d = nc.dram_tensor("kT", [n_heads, 128, L_], bf16, kind="ExternalInput").ap()
    # v1[g, p, lc, d]: e^bias[l] * (V row l | 1), l = lc*128 + p
    v1_d = nc.dram_tensor(
        "v1", [n_heads, 128, n_lc, 129], bf16, kind="ExternalInput"
    ).ap()
    out_d = nc.dram_tensor("out", [n_heads, n_tc, 128, 128], f32, kind="ExternalOutput").ap()

    with tile.TileContext(nc) as tc:
        with ExitStack() as ctx:
            in_pool = ctx.enter_context(tc.tile_pool(name="in_pool", bufs=3))
            e_pool = ctx.enter_context(tc.tile_pool(name="e_pool", bufs=2))
            ep_pool = ctx.enter_context(tc.tile_pool(name="ep_pool", bufs=4))
            s_pool = ctx.enter_context(tc.tile_pool(name="s_pool", bufs=3, space="PSUM"))
            o_pool = ctx.enter_context(tc.tile_pool(name="o_pool", bufs=2, space="PSUM"))

            for g in range(n_heads):
                qT = in_pool.tile([128, T_], bf16, tag="qT", name="qT_sb")
                nc.sync.dma_start(out=qT, in_=qT_d[g])
                kT = in_pool.tile([128, L_], bf16, tag="kT", name="kT_sb")
                for c in range(KT_SPLIT):
                    w = L_ // KT_SPLIT
                    nc.sync.dma_start(
                        out=kT[:, c * w : (c + 1) * w],
                        in_=kT_d[g, :, c * w : (c + 1) * w],
                    )
                v1 = in_pool.tile([128, n_lc, 129], bf16, tag="v1", name="v1_sb")
                for c in range(V1_SPLIT):
                    w = n_lc // V1_SPLIT
                    nc.gpsimd.dma_start(
                        out=v1[:, c * w : (c + 1) * w, :],
                        in_=v1_d[g, :, c * w : (c + 1) * w, :],
                    )

                # Stage 1: E^T[l, t] = exp(scale * (K Q^T)[l, t]), bf16.
                # ACT_W score matmuls land in one 2-bank PSUM tile; a single
                # wide Exp covers them all (amortizes ACT fixed overhead).
                e = e_pool.tile([128, n_lc, T_], bf16, tag="e", name="e_sb")
                for a in range(n_lc // ACT_W):
                    s = s_pool.tile([128, ACT_W, T_], f32, tag="s", name="s_ps")
                    for j in range(ACT_W):
                        lc = a * ACT_W + j
                        nc.tensor.matmul(
                            s[:, j, :],
                            lhsT=kT[:, ts(lc, 128)],
                            rhs=qT,
                            start=True,
                            stop=True,
                        )
                    nc.scalar.activation(
                        out=e[:, a * ACT_W : (a + 1) * ACT_W, :],
                        in_=s,
                        func=mybir.ActivationFunctionType.Exp,
                        scale=SCALE,
                    )

                # Stage 2: accumulate [O*denom | denom] over l-chunks
                os_ = []
                for tci in range(n_tc):
                    o = o_pool.tile([128, 129], f32, tag="o", name="o_ps")
                    os_.append(o)
                for lc in range(n_lc):
                    for tci in range(n_tc):
                        nc.tensor.matmul(
                            os_[tci],
                            lhsT=e[:, lc, ts(tci, 128)],
                            rhs=v1[:, lc, :],
                            start=(lc == 0),
                            stop=(lc == n_lc - 1),
                        )

                # Epilogue: O = (O*denom) / denom
                for tci in range(n_tc):
                    recip = ep_pool.tile([128, 1], f32, tag="recip", name="recip_sb")
                    nc.vector.reciprocal(recip, os_[tci][:, 128:129])
                    ob = ep_pool.tile([128, 128], f32, tag="ob", name="ob_sb")
                    nc.vector.tensor_scalar_mul(ob, os_[tci][:, 0:128], recip)
                    nc.sync.dma_start(out=out_d[g, tci], in_=ob)

    nc.compile()
    return nc


def make_core_inputs(q, k, v, beta, attn_mask):
    """Host prep: fold mask+beta into bias, transpose/tile/cast, shard 8 ways.

    Returns list of 8 in_maps (one per core)."""
    import ml_dtypes

    bf16 = ml_dtypes.bfloat16

    qf = np.ascontiguousarray(q, np.float32).reshape(G, T, D)
    kf = np.ascontiguousarray(k, np.float32).reshape(G, L, D)
    vf = np.ascontiguousarray(v, np.float32).reshape(G, L, D)

    bias = np.zeros((G, L), np.float32)
    bias[:, :LC] = np.asarray(beta, np.float32).reshape(G, LC)
    mask = np.asarray(attn_mask).reshape(G, L)
    # exp(s + b) = exp(s) * e^b: fold e^bias into the [V | 1] operand so the
    # device exp needs no per-partition bias (enables wide ACT tiles). A
    # masked-out l gets e^-inf = 0, zeroing its numerator+denominator terms.
    ebias = np.where(mask, np.exp(bias), np.float32(0.0))

    in_maps = []
    for i in range(NCORES):
        sl = slice(i * GPC, (i + 1) * GPC)
        qT = np.ascontiguousarray(qf[sl].transpose(0, 2, 1)).astype(bf16)
        kT = np.ascontiguousarray(kf[sl].transpose(0, 2, 1)).astype(bf16)
        v1 = np.empty((GPC, L, D + 1), np.float32)
        v1[..., :D] = vf[sl]
        v1[..., D] = 1.0
        v1 *= ebias[sl, :, None]
        v1 = v1.reshape(GPC, N_LC, 128, D + 1).transpose(0, 2, 1, 3)
        in_maps.append(
            {"qT": qT, "kT": kT, "v1": np.ascontiguousarray(v1.astype(bf16))}
        )
    return in_maps


def run_spmd(in_maps, trace=False):
    from concourse import bass_utils

    if "nc" not in _NC_CACHE:
        _NC_CACHE["nc"] = build_nc()
    nc = _NC_CACHE["nc"]
    return bass_utils.run_bass_kernel_spmd(
        nc, in_maps, core_ids=list(range(NCORES)), trace=trace
    )


def kernel(q, k, v, beta, attn_mask):
    res = run_spmd(make_core_inputs(q, k, v, beta, attn_mask))
    out = np.empty((G, T, D), np.float32)
    for i in range(NCORES):
        out[i * GPC : (i + 1) * GPC] = res.results[i]["out"].reshape(GPC, T, D)
    return out.reshape(B, H, T, D)
